# revision 2
# baseline (speedup 1.0000x reference)
"""Trainium2 Bass kernel for nn_EuclideanLoss.

Math (matches the oracle):
    y_t  = transpose(y, (0, 2, 1))                 # [B, N, D]
    pd   = sqrt(sum((x - y_t)^2, axis=-1))         # [B, N]
    dist = mean(pd, axis=0); dist[1:3] *= 1.5
    loss = mean(dist)

Strategy: data-parallel over batch — each of the 8 NeuronCores takes 4
batches and computes its pair distances pd[b, n] on device; the tiny [B, N]
result is gathered to the host, which finishes mean/scale/mean in float64.

The problem is DMA-bound (16MB of input per core).  Loads are laid out for
address-sequential HBM descriptors and issued as few, large transfers:
  * y[b] ([64, 8192] row-major) loads FLAT into [128, 4096]: partition
    p = 2d + nh holds y[d, nh*4096 : (nh+1)*4096] — 16KB descriptors.
  * x[b] loads as [128, 2, 32, 64] = (q, nh, c, d) with
    n = nh*4096 + q*32 + c — one DMA per batch, 8KB descriptors.
    The LAST batch's x is split (8,8,8,4,4 c-columns) so the final
    sub->square->reduce chain after the last byte lands is short.
  * y3 is loaded early (after x0) so batch-3's PE transposes are off the
    tail critical path.
Compute per batch (c-chunks):
  PE   transposes y_v[:, c, :] ([128, 128]) -> PSUM yT[q, c, 2d+nh],
       aligning y to x's n-to-partition map.
  DVE  diff = x - yT  (fp32, 1x mode — fp32 tensor_tensor can't go faster)
  ACT  sq = Square(diff) -> BF16 (rounding of squares is ~0.2% rms, which
       averages out to ~1e-4 on the final scalar; gate is 2e-2)
  DVE  reduce over d -> d2 (bf16 input enables the 2x DVE mode; internal
       accumulation is fp32)
  ACT  per-batch pd = Sqrt(d2) (hidden under the stream; only batch 3's
       runs in the tail), one contiguous store at the end.

Output o[b, p, g, nh, c] = pd[b, nh*4096 + p*32 + g*8 + c]; host undoes it.
"""

import numpy as np

import concourse.bacc as bacc
import concourse.bass as bass
import concourse.mybir as mybir
import concourse.tile as tile
from concourse import masks
from concourse.bass_utils import run_bass_kernel_spmd

B, N, D = 32, 8192, 64
NCORES = 8
BL = B // NCORES        # 4 local batches per core
P = 128                 # SBUF partitions
NH = 2                  # n-halves per batch (partition interleave of y)
CPB = N // NH // P      # 32 consecutive x rows per partition per half
NG = 4                  # c-groups per batch (transpose granularity)
GC = CPB // NG          # 8 columns per group

F32 = mybir.dt.float32
BF16 = mybir.dt.bfloat16

# compute chunks (c-column ranges) per batch; last batch tapers so the
# final chain operates on little data
CHUNKS = [(c, c + GC) for c in range(0, CPB, GC)]
CHUNKS_LAST = [(0, 8), (8, 16), (16, 24), (24, 28), (28, 32)]


def _build() -> bass.Bass:
    # Bacc (not plain Bass): its compile() pass splits sem waits across
    # event-semaphore instructions — TRN2 instructions hold at most one wait,
    # and this walrus build rejects multi-wait instructions outright.
    nc = bacc.Bacc("TRN2", target_bir_lowering=False, debug=False, num_devices=NCORES)
    x_d = nc.dram_tensor("x", [BL, N, D], F32, kind="ExternalInput")
    y_d = nc.dram_tensor("y", [BL, D, N], F32, kind="ExternalInput")
    o_d = nc.dram_tensor("o", [P, BL, NG, NH, GC], F32, kind="ExternalOutput")

    with tile.TileContext(nc) as tc:
        with (
            tc.tile_pool(name="const", bufs=1) as cpool,
            tc.tile_pool(name="io", bufs=4) as iopool,
            tc.tile_pool(name="work", bufs=4) as wpool,
            tc.tile_pool(name="psum", bufs=4, space="PSUM") as ppool,
        ):
            ident = cpool.tile([P, P], F32)
            masks.make_identity(nc, ident[:])
            d2a = cpool.tile([P, BL, NG, NH, GC], F32)
            pda = cpool.tile([P, BL, NG, NH, GC], F32)
            # Warm the Sqrt LUT during the DMA fill so the per-batch sqrts
            # do not stall ~1.3us on a lazy ACT_TABLE_LOAD.
            warm = cpool.tile([P, 1], F32)
            nc.scalar.activation(
                warm[:], ident[:, 0:1], mybir.ActivationFunctionType.Sqrt
            )

            # ---- issue every input DMA up front, few and large ----------
            x_tiles, y_tiles = [], []
            for b in range(BL):
                x_tiles.append(iopool.tile([P, NH, CPB, D], F32, tag="x", name=f"x{b}"))
                y_tiles.append(
                    iopool.tile([P, NH * CPB * D], F32, tag="y", name=f"y{b}")
                )

            def load_y(b):
                nc.sync.dma_start(
                    y_tiles[b], y_d[b].rearrange("d (nh n) -> (d nh) n", nh=NH)
                )

            def load_x(b, c0, c1):
                xsrc = x_d[b].rearrange("(nh q c) d -> q nh c d", nh=NH, c=CPB)
                nc.sync.dma_start(
                    x_tiles[b][:, :, c0:c1, :], xsrc[:, :, c0:c1, :]
                )

            load_y(0)
            load_x(0, 0, CPB)
            load_y(1)
            load_y(3)            # early: batch-3 transposes off the tail path
            load_x(1, 0, CPB)
            load_y(2)
            load_x(2, 0, CPB)
            for c0, c1 in CHUNKS_LAST:
                load_x(3, c0, c1)

            # ---- per-batch compute --------------------------------------
            for b in range(BL):
                # column q of slice c holds n-offset q*32+c within each half
                y_v = y_tiles[b].rearrange("p (q c) -> p c q", c=CPB)
                chunks = CHUNKS_LAST if b == BL - 1 else CHUNKS
                yT = {}
                for g in range(NG):
                    t = ppool.tile([P, GC, P], F32, tag="yT", name=f"yT{b}_{g}")
                    for c in range(GC):
                        nc.tensor.transpose(
                            t[:, c, :], y_v[:, g * GC + c, :], ident[:]
                        )
                    yT[g] = t

                for c0, c1 in chunks:
                    g, gc0 = c0 // GC, c0 % GC
                    w = c1 - c0
                    diff = wpool.tile([P, NH, w, D], F32, tag="diff", name=f"df{b}{c0}")
                    nc.vector.tensor_sub(
                        diff[:],
                        x_tiles[b][:, :, c0:c1, :],
                        yT[g][:, gc0 : gc0 + w, :].rearrange(
                            "p c (d nh) -> p nh c d", nh=NH
                        ),
                    )
                    sq = wpool.tile([P, NH, w, D], BF16, tag="sq", name=f"sq{b}{c0}")
                    nc.scalar.activation(
                        sq[:], diff[:], mybir.ActivationFunctionType.Square
                    )
                    nc.vector.tensor_reduce(
                        d2a[:, b, g, :, gc0 : gc0 + w],
                        sq[:],
                        axis=mybir.AxisListType.X,
                        op=mybir.AluOpType.add,
                    )

                # per-batch sqrt: batches 0-2 hide under the stream; only
                # batch 3's short [P, 64] sqrt sits in the tail
                nc.scalar.activation(
                    pda[:, b], d2a[:, b], mybir.ActivationFunctionType.Sqrt
                )

            # One contiguous store for all batches: per-batch strided stores
            # interleave small descriptors into the input stream.
            nc.sync.dma_start(o_d[:], pda[:])
    nc.finalize()
    return nc


_NC_CACHE: list = []


def _get_program() -> bass.Bass:
    if not _NC_CACHE:
        _NC_CACHE.append(_build())
    return _NC_CACHE[0]


def kernel(x: np.ndarray, y: np.ndarray) -> np.ndarray:
    x = np.ascontiguousarray(np.asarray(x, dtype=np.float32))
    y = np.ascontiguousarray(np.asarray(y, dtype=np.float32))
    assert x.shape == (B, N, D) and y.shape == (B, D, N)

    nc = _get_program()
    in_maps = [
        {"x": x[i * BL : (i + 1) * BL], "y": y[i * BL : (i + 1) * BL]}
        for i in range(NCORES)
    ]
    res = run_bass_kernel_spmd(nc, in_maps, list(range(NCORES)))
    o = np.stack([res.results[i]["o"] for i in range(NCORES)])  # [8, P, BL, NG, NH, GC]
    # o[core, p, b, g, nh, c] = pd[core*BL + b, nh*4096 + p*32 + g*8 + c]
    pd = (
        o.transpose(0, 2, 4, 1, 3, 5)  # (core, b, nh, p, g, c)
        .reshape(B, N)
    )

    dist = pd.mean(axis=0, dtype=np.float64)
    dist[1:3] *= 1.5
    return np.asarray(dist.mean(), dtype=np.float32)


# revision 5
# speedup vs baseline: 1.0353x; 1.0353x over previous
"""Trainium2 Bass kernel for nn_EuclideanLoss.

Math (matches the oracle):
    y_t  = transpose(y, (0, 2, 1))                 # [B, N, D]
    pd   = sqrt(sum((x - y_t)^2, axis=-1))         # [B, N]
    dist = mean(pd, axis=0); dist[1:3] *= 1.5
    loss = mean(dist)

Strategy: data-parallel over batch — each of the 8 NeuronCores takes 4
batches and computes its pair distances pd[b, n] on device; the tiny [B, N]
result is gathered to the host, which finishes mean/scale/mean in float64.

The problem is DMA-bound (16MB of fp32 input per core, stream ~43us at
~390GB/s); every other engine must stay off that critical path:
  * Inputs are cast fp32 -> bf16 during the DMA itself (SWDGE path,
    nc.gpsimd.dma_start) — the HBM read traffic is unchanged, but the
    bf16 pipeline buys: 2x_1p DVE mode for the subtract (all-bf16 packed
    operands), single-pass bf16 PE transposes, and half the SBUF traffic.
    bf16 quantization of x/y adds ~1e-4 relative error on the final
    scalar (gate is 2e-2).
  * y[b] ([64, 8192] row-major) loads FLAT into [128, 4096]: partition
    p = nh*64 + d holds y[d, nh*4096 : (nh+1)*4096] — 16KB-contiguous
    descriptors.  The (nh d) order makes the transposed view's innermost
    d-stride 1, which the 2x_1p subtract requires.
  * x[b] loads as [128, 2, 32, 64] = (q, nh, c, d) with
    n = nh*4096 + q*32 + c — 4KB contiguous descriptors, two DMAs per
    batch.  The LAST batch is split (16,8,4,4 c-columns) so the final
    sub->square->reduce chain after the last byte lands is short; its y
    is loaded early so batch-3 PE transposes are off the tail path.
Compute per batch (c-chunks of 8, last batch 8,8,8,4,4):
  PE   transposes y_v[:, c, :] ([128, 128] bf16) -> PSUM yT[q, c, nh*64+d]
  DVE  diff = x - yT  (all-bf16 packed -> 2x_1p)
  ACT  sq = Square(diff) -> bf16
  DVE  reduce over d -> d2 fp32 (reduce has no fast mode; fp32 accum out)
  ACT  per-batch pd = Sqrt(d2) (hidden under the stream), one contiguous
       fp32 store at the end (sync/HWDGE queue, disjoint from the input
       stream's gpsimd queue).

Output o[b, p, g, nh, c] = pd[b, nh*4096 + p*32 + g*8 + c]; host undoes it.
"""

import numpy as np

import concourse.bacc as bacc
import concourse.bass as bass
import concourse.mybir as mybir
import concourse.tile as tile
from concourse import masks
from concourse.bass_utils import run_bass_kernel_spmd

B, N, D = 32, 8192, 64
NCORES = 8
BL = B // NCORES        # 4 local batches per core
P = 128                 # SBUF partitions
NH = 2                  # n-halves per batch (partition interleave of y)
CPB = N // NH // P      # 32 consecutive x rows per partition per half
NG = 4                  # c-groups per batch (transpose granularity)
GC = CPB // NG          # 8 columns per group

F32 = mybir.dt.float32
BF16 = mybir.dt.bfloat16

# compute chunks (c-column ranges) per batch; last batch tapers so the
# final chain operates on little data
CHUNKS = [(c, c + GC) for c in range(0, CPB, GC)]
CHUNKS_LAST = [(0, 8), (8, 16), (16, 24), (24, 28), (28, 32)]
# DMA pieces for x (coarser than compute chunks; subtile deps connect them)
XPIECES = [(0, 16), (16, 32)]
XPIECES_LAST = [(0, 16), (16, 24), (24, 28), (28, 32)]


def _build() -> bass.Bass:
    # Bacc (not plain Bass): its compile() pass splits sem waits across
    # event-semaphore instructions — TRN2 instructions hold at most one wait,
    # and this walrus build rejects multi-wait instructions outright.
    nc = bacc.Bacc("TRN2", target_bir_lowering=False, debug=False, num_devices=NCORES)
    x_d = nc.dram_tensor("x", [BL, N, D], F32, kind="ExternalInput")
    y_d = nc.dram_tensor("y", [BL, D, N], F32, kind="ExternalInput")
    o_d = nc.dram_tensor("o", [P, BL, NG, NH, GC], F32, kind="ExternalOutput")

    with tile.TileContext(nc) as tc:
        with (
            tc.tile_pool(name="const", bufs=1) as cpool,
            tc.tile_pool(name="io", bufs=4) as iopool,
            tc.tile_pool(name="work", bufs=4) as wpool,
            tc.tile_pool(name="psum", bufs=4, space="PSUM") as ppool,
        ):
            # ---- issue every input DMA up front, cast fp32->bf16 inline --
            x_tiles, y_tiles = [], []
            for b in range(BL):
                x_tiles.append(
                    iopool.tile([P, NH, CPB, D], BF16, tag="x", name=f"x{b}")
                )
                y_tiles.append(
                    iopool.tile([P, NH * CPB * D], BF16, tag="y", name=f"y{b}")
                )

            def load_y(b):
                # partition nh*64+d <- y[d, nh*4096:(nh+1)*4096]; grouping
                # (nh d) isn't expressible in one rearrange, so one DMA per
                # half onto a partition range (both queue together, all 16
                # SDMA ports stay busy across the pair)
                half = N // NH
                for nh in range(NH):
                    nc.gpsimd.dma_start(
                        y_tiles[b][nh * 64 : (nh + 1) * 64, :],
                        y_d[b][:, nh * half : (nh + 1) * half],
                    )

            def load_x(b, c0, c1):
                xsrc = x_d[b].rearrange("(nh q c) d -> q nh c d", nh=NH, c=CPB)
                nc.gpsimd.dma_start(
                    x_tiles[b][:, :, c0:c1, :], xsrc[:, :, c0:c1, :]
                )

            load_y(0)
            for c0, c1 in XPIECES:
                load_x(0, c0, c1)
            load_y(1)
            load_y(3)            # early: batch-3 transposes off the tail path
            for c0, c1 in XPIECES:
                load_x(1, c0, c1)
            load_y(2)
            for c0, c1 in XPIECES:
                load_x(2, c0, c1)
            for c0, c1 in XPIECES_LAST:
                load_x(3, c0, c1)

            ident = cpool.tile([P, P], BF16)
            masks.make_identity(nc, ident[:])
            d2a = cpool.tile([P, BL, NG, NH, GC], F32)
            pda = cpool.tile([P, BL, NG, NH, GC], F32)
            # Warm the Sqrt LUT during the DMA fill so the per-batch sqrts
            # do not stall ~1.3us on a lazy ACT_TABLE_LOAD.
            warm = cpool.tile([P, 1], F32)
            nc.scalar.activation(
                warm[:], ident[:, 0:1], mybir.ActivationFunctionType.Sqrt
            )

            # ---- per-batch compute --------------------------------------
            for b in range(BL):
                # column q of slice c holds n-offset q*32+c within each half
                y_v = y_tiles[b].rearrange("p (q c) -> p c q", c=CPB)
                chunks = CHUNKS_LAST if b == BL - 1 else CHUNKS
                yT = {}
                for g in range(NG):
                    t = ppool.tile([P, GC, P], BF16, tag="yT", name=f"yT{b}_{g}")
                    for c in range(GC):
                        nc.tensor.transpose(
                            t[:, c, :], y_v[:, g * GC + c, :], ident[:]
                        )
                    yT[g] = t

                for c0, c1 in chunks:
                    g, gc0 = c0 // GC, c0 % GC
                    w = c1 - c0
                    diff = wpool.tile([P, NH, w, D], BF16, tag="diff", name=f"df{b}{c0}")
                    nc.vector.tensor_sub(
                        diff[:],
                        x_tiles[b][:, :, c0:c1, :],
                        yT[g][:, gc0 : gc0 + w, :].rearrange(
                            "p c (nh d) -> p nh c d", nh=NH
                        ),
                    )
                    sq = wpool.tile([P, NH, w, D], BF16, tag="sq", name=f"sq{b}{c0}")
                    nc.scalar.activation(
                        sq[:], diff[:], mybir.ActivationFunctionType.Square
                    )
                    nc.vector.tensor_reduce(
                        d2a[:, b, g, :, gc0 : gc0 + w],
                        sq[:],
                        axis=mybir.AxisListType.X,
                        op=mybir.AluOpType.add,
                    )

                # per-batch sqrt: batches 0-2 hide under the stream; only
                # batch 3's short [P, 64] sqrt sits in the tail
                nc.scalar.activation(
                    pda[:, b], d2a[:, b], mybir.ActivationFunctionType.Sqrt
                )

            # One contiguous store for all batches: per-batch strided stores
            # interleave small descriptors into the input stream.
            nc.sync.dma_start(o_d[:], pda[:])
    nc.finalize()
    return nc


_NC_CACHE: list = []


def _get_program() -> bass.Bass:
    if not _NC_CACHE:
        _NC_CACHE.append(_build())
    return _NC_CACHE[0]


def kernel(x: np.ndarray, y: np.ndarray) -> np.ndarray:
    x = np.ascontiguousarray(np.asarray(x, dtype=np.float32))
    y = np.ascontiguousarray(np.asarray(y, dtype=np.float32))
    assert x.shape == (B, N, D) and y.shape == (B, D, N)

    nc = _get_program()
    in_maps = [
        {"x": x[i * BL : (i + 1) * BL], "y": y[i * BL : (i + 1) * BL]}
        for i in range(NCORES)
    ]
    res = run_bass_kernel_spmd(nc, in_maps, list(range(NCORES)))
    o = np.stack([res.results[i]["o"] for i in range(NCORES)])  # [8, P, BL, NG, NH, GC]
    # o[core, p, b, g, nh, c] = pd[core*BL + b, nh*4096 + p*32 + g*8 + c]
    pd = (
        o.transpose(0, 2, 4, 1, 3, 5)  # (core, b, nh, p, g, c)
        .reshape(B, N)
    )

    dist = pd.mean(axis=0, dtype=np.float64)
    dist[1:3] *= 1.5
    return np.asarray(dist.mean(), dtype=np.float32)
